# revision 1
# baseline (speedup 1.0000x reference)
"""Trainium2 Bass kernel for dynamic-filter 4x upsampling (nn_G_61856118997290).

Math: fw = softmax(filt, axis=1) over 343 taps; per color channel c the
output is pixel-shuffle(sum_p patches(x_c)[p] * fw[p, u]) for u in 0..16.

Computed as exp(filt) streams: N_c = sum_p P_c*E, S = sum_p E, out = N_c/S
(softmax normalization folded into one final division on the host).

Sharding: output rows H=128 split 8 ways (16 rows/core). Per core:
 - E-stream: filt slab [2,343,16,16,128] f32 (90MB) -> ACT exp -> bf16
 - patches P (host im2col, bf16) -> DVE multiply -> Z = P*E
 - PE ones-stationary matmuls reduce the 343-tap partition axis into PSUM
   partition groups {0,32,64} (M=32 replicated), 3 chunks accumulated
 - ACT/DVE evacuate PSUM -> SBUF -> DMA to DRAM
 - host: divide by S, pixel-shuffle, concat cores.
"""
import numpy as np
import ml_dtypes

import concourse.bass as bass
import concourse.tile as tile
from concourse import bacc, mybir
from concourse.bass_utils import run_bass_kernel_spmd

F32 = mybir.dt.float32
BF16 = mybir.dt.bfloat16
EXP = mybir.ActivationFunctionType.Exp

B, C, T, H, W = 2, 3, 7, 128, 128
NHB, PAD, UF = 7, 3, 4
U = UF * UF                 # 16 filter output channels
TAPS = T * NHB * NHB        # 343
NCORES = 8
HL = H // NCORES            # 16 output rows per core
PIX = HL * W                # 2048 pixels per (b,u) plane
KP = [128, 128, 87]         # tap chunks on the partition axis
KS = [0, 128, 256]
NBU = B * U                 # 32 (b,u) planes

_CACHED = {}


def _build():
    nc = bacc.Bacc("TRN2", target_bir_lowering=False, debug=False,
                   num_devices=NCORES)
    fslab = nc.dram_tensor("fslab", [B, TAPS, U, PIX], F32,
                           kind="ExternalInput")
    ptin = nc.dram_tensor("ptin", [B, C, TAPS, PIX], BF16,
                          kind="ExternalInput")
    nout = nc.dram_tensor("nout", [B, U, C, PIX], F32, kind="ExternalOutput")
    sout = nc.dram_tensor("sout", [NBU * 4, 512], F32, kind="ExternalOutput")

    with tile.TileContext(nc) as tc:
        with tc.tile_pool(name="cst", bufs=1) as cst, \
             tc.tile_pool(name="sb", bufs=2) as sb, \
             tc.tile_pool(name="zp", bufs=2, space="PSUM") as zp, \
             tc.tile_pool(name="sp", bufs=4, space="PSUM") as sp:
            ones = cst.tile([128, 32], BF16)
            nc.vector.memset(ones[:], 1.0)
            zbias = cst.tile([128, 1], F32)
            nc.vector.memset(zbias[:], 0.0)

            # resident patch tiles: 18 x [128, 2048] bf16 = 72KB/partition
            # (loaded lazily: b=0 during bu 0, b=1 just before bu 16 to keep
            # the DMA queue clear for the E-stream pipeline fill)
            pt = {}

            def load_pt(b, c, k):
                kp = KP[k]
                t_ = cst.tile([128, PIX], BF16, name=f"pt{b}{c}{k}")
                nc.scalar.dma_start(t_[:kp, :], ptin[b, c, KS[k]:KS[k] + kp, :])
                pt[b, c, k] = t_

            sps = None  # current S psum tile, 3 slots (partition groups)
            for bu in range(NBU):
                b, u = bu // U, bu % U
                ebf = []
                for k, kp in enumerate(KP):
                    eraw = sb.tile([128, PIX], F32, tag="eraw", bufs=6,
                                   name=f"eraw{bu}_{k}")
                    nc.sync.dma_start(eraw[:kp, :],
                                      fslab[b, KS[k]:KS[k] + kp, u, :])
                    et = sb.tile([128, PIX], BF16, tag="ebf", bufs=6,
                                 name=f"ebf{bu}_{k}")
                    nc.scalar.activation(et[:kp, :], eraw[:kp, :], EXP,
                                         bias=zbias[:kp, :])
                    ebf.append(et)
                if bu == 0:  # first patch loads after bu0's E-stream DMAs
                    for c in range(C):
                        for k in range(len(KP)):
                            load_pt(0, c, k)

                zps = [zp.tile([128, 1024], F32, tag="zps",
                               name=f"zps{bu}_{h}") for h in range(2)]
                for c in range(C):
                    zt = []
                    for k, kp in enumerate(KP):
                        z_ = sb.tile([128, PIX], BF16, tag="z", bufs=6,
                                     name=f"z{bu}_{c}_{k}")
                        nc.vector.tensor_mul(z_[:kp, :], ebf[k][:kp, :],
                                             pt[b, c, k][:kp, :])
                        zt.append(z_)
                    for g in range(4):
                        half, col = g // 2, g % 2
                        out_ap = zps[half][32 * c:32 * c + 32,
                                           512 * col:512 * (col + 1)]
                        for k, kp in enumerate(KP):
                            nc.tensor.matmul(
                                out_ap, ones[:kp, :],
                                zt[k][:kp, 512 * g:512 * (g + 1)],
                                start=(k == 0), stop=(k == 2))

                for half in range(2):
                    zsb = sb.tile([128, 1024], F32, tag="zsb", bufs=6,
                                  name=f"zsb{bu}_{half}")
                    nc.scalar.copy(zsb[:96, :], zps[half][:96, :])
                    nc.scalar.dma_start(
                        nout[b, u, :, 1024 * half:1024 * (half + 1)],
                        zsb[:96:32, :])

                # S stream: sum_p E, 4 col-groups -> slots j=bu*4+g of [128,512]
                for g in range(4):
                    j = bu * 4 + g
                    r = j % 3
                    if r == 0:
                        sps = sp.tile([128, 512], F32, tag="sps",
                                      name=f"sps{j}")
                    for k, kp in enumerate(KP):
                        nc.tensor.matmul(
                            sps[32 * r:32 * r + 32, :], ones[:kp, :],
                            ebf[k][:kp, 512 * g:512 * (g + 1)],
                            start=(k == 0), stop=(k == 2))
                    if r == 2 or j == NBU * 4 - 1:
                        ns = r + 1
                        ssb = sb.tile([128, 512], F32, tag="ssb", bufs=4,
                                      name=f"ssb{j}")
                        nc.scalar.copy(ssb[:32 * ns, :], sps[:32 * ns, :])
                        nc.scalar.dma_start(sout[j - ns + 1:j + 1, :],
                                          ssb[:32 * ns:32, :])
                # b=1 patch loads at body end: 1 tile/bu, behind the
                # current bu's E-stream DMAs in queue order
                if 6 <= bu < 15:
                    i = bu - 6
                    load_pt(1, i // 3, i % 3)
    nc.compile()
    return nc


def _prep_core(x, filt, g):
    """Per-core inputs: filt h-slab + host im2col patch tiles (bf16)."""
    h0 = g * HL
    fslab = np.ascontiguousarray(
        filt[:, :, :, h0:h0 + HL, :]).reshape(B, TAPS, U, PIX)
    xpad = np.pad(x, ((0, 0), (0, 0), (0, 0), (PAD, PAD), (PAD, PAD)))
    win = np.lib.stride_tricks.sliding_window_view(
        xpad[:, :, :, h0:h0 + HL + 2 * PAD, :], (HL, W), axis=(3, 4))
    # win: [B, C, T, 7, 7, HL, W] indexed [b,c,t,i,j,hh,ww]
    ptin = np.ascontiguousarray(win).reshape(B, C, TAPS, PIX)
    return {"fslab": fslab, "ptin": ptin.astype(ml_dtypes.bfloat16)}


def kernel(x: np.ndarray, filt: np.ndarray) -> np.ndarray:
    x = np.asarray(x, dtype=np.float32)
    filt = np.asarray(filt, dtype=np.float32)
    if "nc" not in _CACHED:
        _CACHED["nc"] = _build()
    nc = _CACHED["nc"]

    in_maps = [_prep_core(x, filt, g) for g in range(NCORES)]
    res = run_bass_kernel_spmd(nc, in_maps, list(range(NCORES)))

    out = np.empty((B, C, H * UF, W * UF), np.float32)
    for g in range(NCORES):
        n = res.results[g]["nout"]                       # [B,U,C,PIX]
        s = res.results[g]["sout"].reshape(B, U, PIX)    # [B,U,PIX]
        t = n / s[:, :, None, :]                         # [B,U,C,PIX]
        t = t.reshape(B, UF, UF, C, HL, W)               # [b,r1,r2,c,h,w]
        t = t.transpose(0, 3, 4, 1, 5, 2)                # [b,c,h,r1,w,r2]
        out[:, :, g * HL * UF:(g + 1) * HL * UF, :] = t.reshape(
            B, C, HL * UF, W * UF)
    return out

